# revision 23
# baseline (speedup 1.0000x reference)
"""Single-head attention (B=4, S=2048, H=1024) on 8 NeuronCores.

Sharding: core c handles batch b=c//2, query half h=c%2 (1024 queries).
Each core computes K/V for its whole batch locally (duplicated across the
2 cores sharing a batch) -- no collectives.

Device kernel (per core, fp16 matmul inputs / fp32 accumulation):
  QT[do,m] = Wq.T @ hs_q.T   (lhsT=Wq block, rhs=hsT)
  KT[do,n] = Wk.T @ hs.T
  V[n,e]   = hs @ Wv          (lhsT=hsT block, rhs=Wv)
  per 128-query block:
    scores[m,n] = QT.T @ KT   (PSUM, accumulate over hidden)
    psum += 32*mask           (DVE, in-place)
    w = exp(psum/32) -> fp16, row-sums via activation accum_out (ScalarE)
    wT = PE-transpose(w)      (128x128 blocks via identity matmul)
    out[m,e] = wT.T @ V       (PSUM accumulate over keys)
    out = out * (1/rowsum) + bv ; DMA to DRAM

Host rolls the key order per core so its own queries are always columns
0..1023 of hsT (keeps the SPMD program identical across cores; softmax and
the output matmul are invariant to key order as long as mask columns are
rolled the same way).
"""

import functools

import numpy as np

B, S, H = 4, 2048, 1024
N_CORES = 8
P = 128
M = S // 2            # queries per core
DO = H // P           # 8  d_out tiles
DI = H // P           # 8  d_in tiles
NT = S // P           # 16 key tiles of 128
NF = S // 512         # 4  key chunks of 512
MB = M // P           # 8  query blocks of 128
EF = H // 512         # 2  out-feature chunks of 512
INV_SCALE = 1.0 / 32.0  # 1/sqrt(H)

TRACE = False


@functools.lru_cache(maxsize=1)
def _build():
    import concourse.bacc as bacc
    import concourse.mybir as mybir
    import concourse.tile as tile

    f32 = mybir.dt.float32
    f16 = mybir.dt.float16
    Exp = mybir.ActivationFunctionType.Exp
    mult = mybir.AluOpType.mult
    add = mybir.AluOpType.add
    AX = mybir.AxisListType.X

    nc = bacc.Bacc(
        "TRN2", target_bir_lowering=False, debug=False, num_devices=N_CORES
    )

    hsT_d = nc.dram_tensor("hsT", [H, S], f16, kind="ExternalInput")
    wq_d = nc.dram_tensor("wq", [H, H], f16, kind="ExternalInput")
    wk_d = nc.dram_tensor("wk", [H, H], f16, kind="ExternalInput")
    wv_d = nc.dram_tensor("wv", [H, H], f16, kind="ExternalInput")
    bq_d = nc.dram_tensor("bq", [P, DO], f32, kind="ExternalInput")
    bk_d = nc.dram_tensor("bk", [P, DO], f32, kind="ExternalInput")
    bv_d = nc.dram_tensor("bv", [P, H], f32, kind="ExternalInput")
    mask_d = nc.dram_tensor("mask", [S, M], f16, kind="ExternalInput")
    out_d = nc.dram_tensor("out", [M, H], f32, kind="ExternalOutput")

    with (
        tile.TileContext(nc) as tc,
        tc.tile_pool(name="const", bufs=1) as cpool,
        tc.tile_pool(name="acts", bufs=1) as apool,
        tc.tile_pool(name="wexp", bufs=2) as wpool,
        tc.tile_pool(name="stream", bufs=3) as spool,
        tc.tile_pool(name="outp", bufs=1) as opool,
        tc.tile_pool(name="ps", bufs=2, space="PSUM") as ps,
        tc.tile_pool(name="pso", bufs=4, space="PSUM") as pso,
        tc.tile_pool(name="pden", bufs=2, space="PSUM") as pden,
    ):
        # ---- constant loads ----
        # Split the big loads into quarters/halves: separate DMAs land on
        # separate queues and overlap, shortening the startup critical path.
        hsT_sb = cpool.tile([P, DI, S], f16, tag="hsT")
        hsT_r = hsT_d.ap().rearrange("(o p) f -> p o f", p=P)
        wv_sb = cpool.tile([P, DI, H], f16, tag="wv")
        wv_r = wv_d.ap().rearrange("(o p) f -> p o f", p=P)
        wk_sb = cpool.tile([P, DI, H], f16, tag="wk")
        wk_r = wk_d.ap().rearrange("(o p) f -> p o f", p=P)
        wq_sb = cpool.tile([P, DI, H], f16, tag="wq")
        wq_r = wq_d.ap().rearrange("(o p) f -> p o f", p=P)
        # arrival order ~ emission order: hsT chunk0 + all weights first so
        # V/KT/QT on chunk0 can run while the rest of hsT streams in.
        nc.sync.dma_start(hsT_sb[:, :, 0:512], hsT_r[:, :, 0:512])
        nc.sync.dma_start(wv_sb[:, :, 0:512], wv_r[:, :, 0:512])
        nc.sync.dma_start(wv_sb[:, :, 512:1024], wv_r[:, :, 512:1024])
        for i in range(2):
            nc.sync.dma_start(wk_sb[:, :, 512 * i:512 * (i + 1)], wk_r[:, :, 512 * i:512 * (i + 1)])
        nc.sync.dma_start(hsT_sb[:, :, 512:1024], hsT_r[:, :, 512:1024])
        nc.sync.dma_start(hsT_sb[:, :, 1024:1536], hsT_r[:, :, 1024:1536])
        for i in range(2):
            nc.sync.dma_start(wq_sb[:, :, 512 * i:512 * (i + 1)], wq_r[:, :, 512 * i:512 * (i + 1)])
        nc.sync.dma_start(hsT_sb[:, :, 1536:2048], hsT_r[:, :, 1536:2048])
        bq_sb = cpool.tile([P, DO], f32, tag="bq")
        nc.sync.dma_start(bq_sb[:], bq_d.ap())
        bk_sb = cpool.tile([P, DO], f32, tag="bk")
        nc.sync.dma_start(bk_sb[:], bk_d.ap())
        bv_sb = cpool.tile([P, H], f32, tag="bv")
        nc.sync.dma_start(bv_sb[:], bv_d.ap())
        ones_sb = cpool.tile([P, 1], f16, tag="ones")
        nc.any.memset(ones_sb[:], 1.0)

        QT_sb = apool.tile([P, DO, M], f16, tag="qt")
        KT_sb = apool.tile([P, DO, S], f16, tag="kt")
        V_sb = apool.tile([P, NT, H], f16, tag="v")

        # ---- projections (chunk-major: consume each 512-col hsT chunk
        # for V, KT and QT work as soon as it lands) ----
        for c in range(4):
            # V rows 4c..4c+3: lhsT = hsT[di, nt-block], rhs = Wv[di, e-chunk]
            for nt in range(4 * c, 4 * c + 4):
                pv = [ps.tile([P, 512], f32, tag="ps", name=f"pv{nt}_{i}") for i in range(EF)]
                for di in range(DI):
                    lh = hsT_sb[:, di, nt * P:(nt + 1) * P]
                    for ef in range(EF):
                        nc.tensor.matmul(
                            pv[ef], lh, wv_sb[:, di, ef * 512:(ef + 1) * 512],
                            start=(di == 0), stop=(di == DI - 1),
                        )
                for ef in range(EF):
                    nc.scalar.copy(V_sb[:, nt, ef * 512:(ef + 1) * 512], pv[ef])
            # KT columns of this chunk: lhsT = Wk[di, o-block], rhs = hsT chunk
            for o in range(DO):
                pk = ps.tile([P, 512], f32, tag="ps", name=f"pk{o}_{c}")
                for di in range(DI):
                    nc.tensor.matmul(
                        pk, wk_sb[:, di, o * P:(o + 1) * P],
                        hsT_sb[:, di, c * 512:(c + 1) * 512],
                        start=(di == 0), stop=(di == DI - 1),
                    )
                nc.vector.tensor_scalar_add(
                    KT_sb[:, o, c * 512:(c + 1) * 512], pk, bk_sb[:, o:o + 1]
                )
            # QT columns (first two chunks only: queries are cols 0..1023)
            if c < 2:
                for o in range(DO):
                    pq = ps.tile([P, 512], f32, tag="ps", name=f"pq{o}_{c}")
                    for di in range(DI):
                        nc.tensor.matmul(
                            pq, wq_sb[:, di, o * P:(o + 1) * P],
                            hsT_sb[:, di, c * 512:(c + 1) * 512],
                            start=(di == 0), stop=(di == DI - 1),
                        )
                    nc.vector.tensor_scalar_add(
                        QT_sb[:, o, c * 512:(c + 1) * 512], pq, bq_sb[:, o:o + 1]
                    )

        # ---- attention: transposed scores, 512-query phases ----
        # scoresT[n, m] = KT.T @ QT per 128-key tile; exp feeds the output
        # matmul directly as lhsT (no transposes). Row sums come from an
        # extra 1-column matmul against a ones vector, sharing the lhsT load.
        QM = 512  # query columns per phase
        for q in range(M // QM):
            expT = wpool.tile([P, NT, QM], f16, tag="expT", name=f"expT{q}")
            for nt in range(NT):
                sc = ps.tile([P, QM], f32, tag="ps", name=f"sc{q}_{nt}")
                for d in range(DO):
                    nc.tensor.matmul(
                        sc, KT_sb[:, d, nt * P:(nt + 1) * P],
                        QT_sb[:, d, q * QM:(q + 1) * QM],
                        start=(d == 0), stop=(d == DO - 1),
                    )
                mtile = spool.tile([P, QM], f16, tag="mask", name=f"mt{q}_{nt}")
                nc.sync.dma_start(
                    mtile[:],
                    mask_d.ap()[nt * P:(nt + 1) * P, q * QM:(q + 1) * QM],
                )
                nc.vector.tensor_add(sc, sc, mtile[:])
                nc.scalar.activation(
                    expT[:, nt, :], sc, Exp, scale=INV_SCALE
                )
            for j in range(QM // P):
                mb = (QM // P) * q + j
                po = [
                    pso.tile([P, 512], f32, tag="pso", name=f"po{mb}_{i}")
                    for i in range(EF)
                ]
                pd = pden.tile([P, 1], f32, tag="pden", name=f"pd{mb}")
                for nt in range(NT):
                    lh = expT[:, nt, j * P:(j + 1) * P]
                    for ef in range(EF):
                        nc.tensor.matmul(
                            po[ef], lh, V_sb[:, nt, ef * 512:(ef + 1) * 512],
                            start=(nt == 0), stop=(nt == NT - 1),
                        )
                    nc.tensor.matmul(
                        pd, lh, ones_sb[:, 0:1],
                        start=(nt == 0), stop=(nt == NT - 1),
                    )
                recip = spool.tile([P, 1], f32, tag="recip", name=f"rc{mb}")
                nc.vector.reciprocal(recip[:], pd)
                osb = opool.tile([P, H], f32, tag="osb", name=f"osb{mb}")
                for ef in range(EF):
                    nc.vector.scalar_tensor_tensor(
                        osb[:, ef * 512:(ef + 1) * 512], po[ef], recip[:],
                        bv_sb[:, ef * 512:(ef + 1) * 512],
                        op0=mult, op1=add,
                    )
                nc.sync.dma_start(out_d.ap()[mb * P:(mb + 1) * P, :], osb[:])

    nc.compile()
    return nc


def kernel(hidden_states, attention_mask, Wq, bq, Wk, bk, Wv, bv):
    from concourse.bass_utils import run_bass_kernel_spmd

    nc = _build()

    hs = np.asarray(hidden_states, dtype=np.float32)
    mask = np.asarray(attention_mask, dtype=np.float32)
    wq16 = np.ascontiguousarray(np.asarray(Wq, dtype=np.float32).astype(np.float16))
    wk16 = np.ascontiguousarray(np.asarray(Wk, dtype=np.float32).astype(np.float16))
    wv16 = np.ascontiguousarray(np.asarray(Wv, dtype=np.float32).astype(np.float16))
    bq32 = np.ascontiguousarray(
        np.asarray(bq, dtype=np.float32).reshape(DO, P).T
    )
    bk32 = np.ascontiguousarray(
        np.asarray(bk, dtype=np.float32).reshape(DO, P).T
    )
    bv32 = np.ascontiguousarray(
        np.broadcast_to(np.asarray(bv, dtype=np.float32).reshape(1, H), (P, H))
    )

    in_maps = []
    for c in range(N_CORES):
        b, h = divmod(c, 2)
        qs = h * M
        if h == 0:
            order = np.arange(S)
        else:
            order = np.concatenate([np.arange(M, S), np.arange(0, M)])
        hs_r = hs[b][order]  # [S, H], own queries first
        hsT_c = np.ascontiguousarray(hs_r.T.astype(np.float16))  # [H, S]
        mask_c = np.ascontiguousarray(
            (32.0 * mask[b, qs:qs + M][:, order]).T
        ).astype(np.float16)
        in_maps.append(
            {
                "hsT": hsT_c,
                "wq": wq16,
                "wk": wk16,
                "wv": wv16,
                "bq": bq32,
                "bk": bk32,
                "bv": bv32,
                "mask": mask_c,
            }
        )

    res = run_bass_kernel_spmd(
        nc, in_maps, core_ids=list(range(N_CORES)), trace=TRACE
    )
    if TRACE:
        kernel.last_results = res

    out = np.empty((B, S, H), dtype=np.float32)
    for c in range(N_CORES):
        b, h = divmod(c, 2)
        out[b, h * M:(h + 1) * M] = res.results[c]["out"]
    return out


# revision 24
# speedup vs baseline: 1.0324x; 1.0324x over previous
"""Single-head attention (B=4, S=2048, H=1024) on 8 NeuronCores.

Sharding: core c handles batch b=c//2, query half h=c%2 (1024 queries).
Each core computes K/V for its whole batch locally (duplicated across the
2 cores sharing a batch) -- no collectives.

Device kernel (per core, fp16 matmul inputs / fp32 accumulation):
  QT[do,m] = Wq.T @ hs_q.T   (lhsT=Wq block, rhs=hsT)
  KT[do,n] = Wk.T @ hs.T
  V[n,e]   = hs @ Wv          (lhsT=hsT block, rhs=Wv)
  per 128-query block:
    scores[m,n] = QT.T @ KT   (PSUM, accumulate over hidden)
    psum += 32*mask           (DVE, in-place)
    w = exp(psum/32) -> fp16, row-sums via activation accum_out (ScalarE)
    wT = PE-transpose(w)      (128x128 blocks via identity matmul)
    out[m,e] = wT.T @ V       (PSUM accumulate over keys)
    out = out * (1/rowsum) + bv ; DMA to DRAM

Host rolls the key order per core so its own queries are always columns
0..1023 of hsT (keeps the SPMD program identical across cores; softmax and
the output matmul are invariant to key order as long as mask columns are
rolled the same way).
"""

import functools

import numpy as np

B, S, H = 4, 2048, 1024
N_CORES = 8
P = 128
M = S // 2            # queries per core
DO = H // P           # 8  d_out tiles
DI = H // P           # 8  d_in tiles
NT = S // P           # 16 key tiles of 128
NF = S // 512         # 4  key chunks of 512
MB = M // P           # 8  query blocks of 128
EF = H // 512         # 2  out-feature chunks of 512
INV_SCALE = 1.0 / 32.0  # 1/sqrt(H)

TRACE = False


@functools.lru_cache(maxsize=1)
def _build():
    import concourse.bacc as bacc
    import concourse.mybir as mybir
    import concourse.tile as tile

    f32 = mybir.dt.float32
    f16 = mybir.dt.float16
    Exp = mybir.ActivationFunctionType.Exp
    mult = mybir.AluOpType.mult
    add = mybir.AluOpType.add
    AX = mybir.AxisListType.X

    nc = bacc.Bacc(
        "TRN2", target_bir_lowering=False, debug=False, num_devices=N_CORES
    )

    hsT_d = nc.dram_tensor("hsT", [H, S], f16, kind="ExternalInput")
    wq_d = nc.dram_tensor("wq", [H, H], f16, kind="ExternalInput")
    wk_d = nc.dram_tensor("wk", [H, H], f16, kind="ExternalInput")
    wv_d = nc.dram_tensor("wv", [H, H], f16, kind="ExternalInput")
    bq_d = nc.dram_tensor("bq", [P, DO], f32, kind="ExternalInput")
    bk_d = nc.dram_tensor("bk", [P, DO], f32, kind="ExternalInput")
    bv_d = nc.dram_tensor("bv", [P, H], f32, kind="ExternalInput")
    mask_d = nc.dram_tensor("mask", [S, M], f16, kind="ExternalInput")
    out_d = nc.dram_tensor("out", [M, H], f32, kind="ExternalOutput")

    with (
        tile.TileContext(nc) as tc,
        tc.tile_pool(name="const", bufs=1) as cpool,
        tc.tile_pool(name="acts", bufs=1) as apool,
        tc.tile_pool(name="wexp", bufs=2) as wpool,
        tc.tile_pool(name="stream", bufs=3) as spool,
        tc.tile_pool(name="outp", bufs=1) as opool,
        tc.tile_pool(name="ps", bufs=3, space="PSUM") as ps,
        tc.tile_pool(name="pso", bufs=3, space="PSUM") as pso,
        tc.tile_pool(name="pden", bufs=2, space="PSUM") as pden,
    ):
        # ---- constant loads ----
        # Split the big loads into quarters/halves: separate DMAs land on
        # separate queues and overlap, shortening the startup critical path.
        hsT_sb = cpool.tile([P, DI, S], f16, tag="hsT")
        hsT_r = hsT_d.ap().rearrange("(o p) f -> p o f", p=P)
        wv_sb = cpool.tile([P, DI, H], f16, tag="wv")
        wv_r = wv_d.ap().rearrange("(o p) f -> p o f", p=P)
        wk_sb = cpool.tile([P, DI, H], f16, tag="wk")
        wk_r = wk_d.ap().rearrange("(o p) f -> p o f", p=P)
        wq_sb = cpool.tile([P, DI, H], f16, tag="wq")
        wq_r = wq_d.ap().rearrange("(o p) f -> p o f", p=P)
        # arrival order ~ emission order: hsT chunk0 + all weights first so
        # V/KT/QT on chunk0 can run while the rest of hsT streams in.
        nc.sync.dma_start(hsT_sb[:, :, 0:512], hsT_r[:, :, 0:512])
        nc.sync.dma_start(wv_sb[:, :, 0:512], wv_r[:, :, 0:512])
        nc.sync.dma_start(wv_sb[:, :, 512:1024], wv_r[:, :, 512:1024])
        for i in range(2):
            nc.sync.dma_start(wk_sb[:, :, 512 * i:512 * (i + 1)], wk_r[:, :, 512 * i:512 * (i + 1)])
        nc.sync.dma_start(hsT_sb[:, :, 512:1024], hsT_r[:, :, 512:1024])
        nc.sync.dma_start(hsT_sb[:, :, 1024:1536], hsT_r[:, :, 1024:1536])
        for i in range(2):
            nc.sync.dma_start(wq_sb[:, :, 512 * i:512 * (i + 1)], wq_r[:, :, 512 * i:512 * (i + 1)])
        nc.sync.dma_start(hsT_sb[:, :, 1536:2048], hsT_r[:, :, 1536:2048])
        bq_sb = cpool.tile([P, DO], f32, tag="bq")
        nc.sync.dma_start(bq_sb[:], bq_d.ap())
        bk_sb = cpool.tile([P, DO], f32, tag="bk")
        nc.sync.dma_start(bk_sb[:], bk_d.ap())
        bv_sb = cpool.tile([P, H], f32, tag="bv")
        nc.sync.dma_start(bv_sb[:], bv_d.ap())
        ones_sb = cpool.tile([P, 1], f16, tag="ones")
        nc.any.memset(ones_sb[:], 1.0)

        QT_sb = apool.tile([P, DO, M], f16, tag="qt")
        KT_sb = apool.tile([P, DO, S], f16, tag="kt")
        V_sb = apool.tile([P, NT, H], f16, tag="v")

        # ---- projections (chunk-major: consume each 512-col hsT chunk
        # for V, KT and QT work as soon as it lands) ----
        for c in range(4):
            # V rows 4c..4c+3: lhsT = hsT[di, nt-block], rhs = Wv[di, e-chunk]
            for nt in range(4 * c, 4 * c + 4):
                pv = [ps.tile([P, 512], f32, tag="ps", name=f"pv{nt}_{i}") for i in range(EF)]
                for di in range(DI):
                    lh = hsT_sb[:, di, nt * P:(nt + 1) * P]
                    for ef in range(EF):
                        nc.tensor.matmul(
                            pv[ef], lh, wv_sb[:, di, ef * 512:(ef + 1) * 512],
                            start=(di == 0), stop=(di == DI - 1),
                        )
                for ef in range(EF):
                    nc.scalar.copy(V_sb[:, nt, ef * 512:(ef + 1) * 512], pv[ef])
            # KT columns of this chunk: lhsT = Wk[di, o-block], rhs = hsT chunk
            for o in range(DO):
                pk = ps.tile([P, 512], f32, tag="ps", name=f"pk{o}_{c}")
                for di in range(DI):
                    nc.tensor.matmul(
                        pk, wk_sb[:, di, o * P:(o + 1) * P],
                        hsT_sb[:, di, c * 512:(c + 1) * 512],
                        start=(di == 0), stop=(di == DI - 1),
                    )
                nc.vector.tensor_scalar_add(
                    KT_sb[:, o, c * 512:(c + 1) * 512], pk, bk_sb[:, o:o + 1]
                )
            # QT columns (first two chunks only: queries are cols 0..1023)
            if c < 2:
                for o in range(DO):
                    pq = ps.tile([P, 512], f32, tag="ps", name=f"pq{o}_{c}")
                    for di in range(DI):
                        nc.tensor.matmul(
                            pq, wq_sb[:, di, o * P:(o + 1) * P],
                            hsT_sb[:, di, c * 512:(c + 1) * 512],
                            start=(di == 0), stop=(di == DI - 1),
                        )
                    nc.vector.tensor_scalar_add(
                        QT_sb[:, o, c * 512:(c + 1) * 512], pq, bq_sb[:, o:o + 1]
                    )

        # ---- attention: transposed scores, 512-query phases ----
        # scoresT[n, m] = KT.T @ QT per 128-key tile; exp feeds the output
        # matmul directly as lhsT (no transposes). Row sums come from an
        # extra 1-column matmul against a ones vector, sharing the lhsT load.
        QM = 512  # query columns per phase
        for q in range(M // QM):
            expT = wpool.tile([P, NT, QM], f16, tag="expT", name=f"expT{q}")
            for nt in range(NT):
                sc = ps.tile([P, QM], f32, tag="ps", name=f"sc{q}_{nt}")
                for d in range(DO):
                    nc.tensor.matmul(
                        sc, KT_sb[:, d, nt * P:(nt + 1) * P],
                        QT_sb[:, d, q * QM:(q + 1) * QM],
                        start=(d == 0), stop=(d == DO - 1),
                    )
                mtile = spool.tile([P, QM], f16, tag="mask", name=f"mt{q}_{nt}")
                nc.sync.dma_start(
                    mtile[:],
                    mask_d.ap()[nt * P:(nt + 1) * P, q * QM:(q + 1) * QM],
                )
                nc.vector.tensor_add(sc, sc, mtile[:])
                nc.scalar.activation(
                    expT[:, nt, :], sc, Exp, scale=INV_SCALE
                )
            for j in range(QM // P):
                mb = (QM // P) * q + j
                po = [
                    pso.tile([P, 512], f32, tag="pso", name=f"po{mb}_{i}")
                    for i in range(EF)
                ]
                pd = pden.tile([P, 1], f32, tag="pden", name=f"pd{mb}")
                for nt in range(NT):
                    lh = expT[:, nt, j * P:(j + 1) * P]
                    for ef in range(EF):
                        nc.tensor.matmul(
                            po[ef], lh, V_sb[:, nt, ef * 512:(ef + 1) * 512],
                            start=(nt == 0), stop=(nt == NT - 1),
                        )
                    nc.tensor.matmul(
                        pd, lh, ones_sb[:, 0:1],
                        start=(nt == 0), stop=(nt == NT - 1),
                    )
                recip = spool.tile([P, 1], f32, tag="recip", name=f"rc{mb}")
                nc.vector.reciprocal(recip[:], pd)
                osb = opool.tile([P, H], f32, tag="osb", name=f"osb{mb}")
                for ef in range(EF):
                    nc.vector.scalar_tensor_tensor(
                        osb[:, ef * 512:(ef + 1) * 512], po[ef], recip[:],
                        bv_sb[:, ef * 512:(ef + 1) * 512],
                        op0=mult, op1=add,
                    )
                nc.sync.dma_start(out_d.ap()[mb * P:(mb + 1) * P, :], osb[:])

    nc.compile()
    return nc


def kernel(hidden_states, attention_mask, Wq, bq, Wk, bk, Wv, bv):
    from concourse.bass_utils import run_bass_kernel_spmd

    nc = _build()

    hs = np.asarray(hidden_states, dtype=np.float32)
    mask = np.asarray(attention_mask, dtype=np.float32)
    wq16 = np.ascontiguousarray(np.asarray(Wq, dtype=np.float32).astype(np.float16))
    wk16 = np.ascontiguousarray(np.asarray(Wk, dtype=np.float32).astype(np.float16))
    wv16 = np.ascontiguousarray(np.asarray(Wv, dtype=np.float32).astype(np.float16))
    bq32 = np.ascontiguousarray(
        np.asarray(bq, dtype=np.float32).reshape(DO, P).T
    )
    bk32 = np.ascontiguousarray(
        np.asarray(bk, dtype=np.float32).reshape(DO, P).T
    )
    bv32 = np.ascontiguousarray(
        np.broadcast_to(np.asarray(bv, dtype=np.float32).reshape(1, H), (P, H))
    )

    in_maps = []
    for c in range(N_CORES):
        b, h = divmod(c, 2)
        qs = h * M
        if h == 0:
            order = np.arange(S)
        else:
            order = np.concatenate([np.arange(M, S), np.arange(0, M)])
        hs_r = hs[b][order]  # [S, H], own queries first
        hsT_c = np.ascontiguousarray(hs_r.T.astype(np.float16))  # [H, S]
        mask_c = np.ascontiguousarray(
            (32.0 * mask[b, qs:qs + M][:, order]).T
        ).astype(np.float16)
        in_maps.append(
            {
                "hsT": hsT_c,
                "wq": wq16,
                "wk": wk16,
                "wv": wv16,
                "bq": bq32,
                "bk": bk32,
                "bv": bv32,
                "mask": mask_c,
            }
        )

    res = run_bass_kernel_spmd(
        nc, in_maps, core_ids=list(range(N_CORES)), trace=TRACE
    )
    if TRACE:
        kernel.last_results = res

    out = np.empty((B, S, H), dtype=np.float32)
    for c in range(N_CORES):
        b, h = divmod(c, 2)
        out[b, h * M:(h + 1) * M] = res.results[c]["out"]
    return out


# revision 25
# speedup vs baseline: 1.3299x; 1.2881x over previous
"""Single-head attention (B=4, S=2048, H=1024) on 8 NeuronCores.

Sharding: core c handles batch b=c//2, query half h=c%2 (1024 queries).
Each core computes K/V for its whole batch locally (duplicated across the
2 cores sharing a batch) -- no collectives.

Device kernel (per core, fp16 matmul inputs / fp32 accumulation):
  QT[do,m] = Wq.T @ hs_q.T   (lhsT=Wq block, rhs=hsT)
  KT[do,n] = Wk.T @ hs.T
  V[n,e]   = hs @ Wv          (lhsT=hsT block, rhs=Wv)
  per 128-query block:
    scores[m,n] = QT.T @ KT   (PSUM, accumulate over hidden)
    psum += 32*mask           (DVE, in-place)
    w = exp(psum/32) -> fp16, row-sums via activation accum_out (ScalarE)
    wT = PE-transpose(w)      (128x128 blocks via identity matmul)
    out[m,e] = wT.T @ V       (PSUM accumulate over keys)
    out = out * (1/rowsum) + bv ; DMA to DRAM

Host rolls the key order per core so its own queries are always columns
0..1023 of hsT (keeps the SPMD program identical across cores; softmax and
the output matmul are invariant to key order as long as mask columns are
rolled the same way).
"""

import functools

import numpy as np

B, S, H = 4, 2048, 1024
N_CORES = 8
P = 128
M = S // 2            # queries per core
DO = H // P           # 8  d_out tiles
DI = H // P           # 8  d_in tiles
NT = S // P           # 16 key tiles of 128
NF = S // 512         # 4  key chunks of 512
MB = M // P           # 8  query blocks of 128
EF = H // 512         # 2  out-feature chunks of 512
INV_SCALE = 1.0 / 32.0  # 1/sqrt(H)

TRACE = False


@functools.lru_cache(maxsize=1)
def _build():
    import concourse.bacc as bacc
    import concourse.mybir as mybir
    import concourse.tile as tile

    f32 = mybir.dt.float32
    f16 = mybir.dt.float16
    Exp = mybir.ActivationFunctionType.Exp
    mult = mybir.AluOpType.mult
    add = mybir.AluOpType.add
    AX = mybir.AxisListType.X

    nc = bacc.Bacc(
        "TRN2", target_bir_lowering=False, debug=False, num_devices=N_CORES
    )

    hsT_d = nc.dram_tensor("hsT", [H, S], f16, kind="ExternalInput")
    g_d = nc.dram_tensor("g", [H, H], f16, kind="ExternalInput")
    wv_d = nc.dram_tensor("wv", [H, H], f16, kind="ExternalInput")
    bv_d = nc.dram_tensor("bv", [P, H], f32, kind="ExternalInput")
    mask_d = nc.dram_tensor("mask", [S, M], f16, kind="ExternalInput")
    out_d = nc.dram_tensor("out", [M, H], f32, kind="ExternalOutput")

    with (
        tile.TileContext(nc) as tc,
        tc.tile_pool(name="const", bufs=1) as cpool,
        tc.tile_pool(name="acts", bufs=1) as apool,
        tc.tile_pool(name="wexp", bufs=2) as wpool,
        tc.tile_pool(name="stream", bufs=3) as spool,
        tc.tile_pool(name="outp", bufs=1) as opool,
        tc.tile_pool(name="ps", bufs=3, space="PSUM") as ps,
        tc.tile_pool(name="pso", bufs=3, space="PSUM") as pso,
        tc.tile_pool(name="pden", bufs=2, space="PSUM") as pden,
    ):
        # ---- constant loads ----
        # Split the big loads into quarters/halves: separate DMAs land on
        # separate queues and overlap, shortening the startup critical path.
        hsT_sb = cpool.tile([P, DI, S], f16, tag="hsT")
        hsT_r = hsT_d.ap().rearrange("(o p) f -> p o f", p=P)
        g_sb = cpool.tile([P, DI, H], f16, tag="g")
        g_r = g_d.ap().rearrange("(o p) f -> p o f", p=P)
        wv_sb = cpool.tile([P, DI, H], f16, tag="wv")
        wv_r = wv_d.ap().rearrange("(o p) f -> p o f", p=P)
        nc.sync.dma_start(hsT_sb[:, :, 0:512], hsT_r[:, :, 0:512])
        nc.sync.dma_start(wv_sb[:, :, 0:512], wv_r[:, :, 0:512])
        nc.sync.dma_start(wv_sb[:, :, 512:1024], wv_r[:, :, 512:1024])
        for i in range(2):
            nc.sync.dma_start(g_sb[:, :, 512 * i:512 * (i + 1)], g_r[:, :, 512 * i:512 * (i + 1)])
        for i in range(1, 4):
            nc.sync.dma_start(
                hsT_sb[:, :, 512 * i:512 * (i + 1)], hsT_r[:, :, 512 * i:512 * (i + 1)]
            )
        bv_sb = cpool.tile([P, H], f32, tag="bv")
        nc.sync.dma_start(bv_sb[:], bv_d.ap())
        ones_sb = cpool.tile([P, 1], f16, tag="ones")
        nc.any.memset(ones_sb[:], 1.0)

        QT_sb = apool.tile([P, DO, M], f16, tag="qt")
        V_sb = apool.tile([P, NT, H], f16, tag="v")

        # ---- projections (chunk-major: consume each 512-col hsT chunk
        # for V, KT and QT work as soon as it lands) ----
        for c in range(4):
            # V rows 4c..4c+3: lhsT = hsT[di, nt-block], rhs = Wv[di, e-chunk]
            for nt in range(4 * c, 4 * c + 4):
                pv = [ps.tile([P, 512], f32, tag="ps", name=f"pv{nt}_{i}") for i in range(EF)]
                for di in range(DI):
                    lh = hsT_sb[:, di, nt * P:(nt + 1) * P]
                    for ef in range(EF):
                        nc.tensor.matmul(
                            pv[ef], lh, wv_sb[:, di, ef * 512:(ef + 1) * 512],
                            start=(di == 0), stop=(di == DI - 1),
                        )
                for ef in range(EF):
                    nc.scalar.copy(V_sb[:, nt, ef * 512:(ef + 1) * 512], pv[ef])
            # QT columns (first two chunks only: queries are cols 0..1023)
            if c < 2:
                for o in range(DO):
                    pq = ps.tile([P, 512], f32, tag="ps", name=f"pq{o}_{c}")
                    for di in range(DI):
                        nc.tensor.matmul(
                            pq, g_sb[:, di, o * P:(o + 1) * P],
                            hsT_sb[:, di, c * 512:(c + 1) * 512],
                            start=(di == 0), stop=(di == DI - 1),
                        )
                    nc.vector.tensor_copy(
                        QT_sb[:, o, c * 512:(c + 1) * 512], pq
                    )

        # ---- attention: transposed scores, 512-query phases ----
        # scoresT[n, m] = KT.T @ QT per 128-key tile; exp feeds the output
        # matmul directly as lhsT (no transposes). Row sums come from an
        # extra 1-column matmul against a ones vector, sharing the lhsT load.
        QM = 512  # query columns per phase
        for q in range(M // QM):
            expT = wpool.tile([P, NT, QM], f16, tag="expT", name=f"expT{q}")
            for nt in range(NT):
                sc = ps.tile([P, QM], f32, tag="ps", name=f"sc{q}_{nt}")
                for d in range(DO):
                    nc.tensor.matmul(
                        sc, hsT_sb[:, d, nt * P:(nt + 1) * P],
                        QT_sb[:, d, q * QM:(q + 1) * QM],
                        start=(d == 0), stop=(d == DO - 1),
                    )
                mtile = spool.tile([P, QM], f16, tag="mask", name=f"mt{q}_{nt}")
                nc.sync.dma_start(
                    mtile[:],
                    mask_d.ap()[nt * P:(nt + 1) * P, q * QM:(q + 1) * QM],
                )
                nc.vector.tensor_add(sc, sc, mtile[:])
                nc.scalar.activation(
                    expT[:, nt, :], sc, Exp, scale=INV_SCALE
                )
            for j in range(QM // P):
                mb = (QM // P) * q + j
                po = [
                    pso.tile([P, 512], f32, tag="pso", name=f"po{mb}_{i}")
                    for i in range(EF)
                ]
                pd = pden.tile([P, 1], f32, tag="pden", name=f"pd{mb}")
                for nt in range(NT):
                    lh = expT[:, nt, j * P:(j + 1) * P]
                    for ef in range(EF):
                        nc.tensor.matmul(
                            po[ef], lh, V_sb[:, nt, ef * 512:(ef + 1) * 512],
                            start=(nt == 0), stop=(nt == NT - 1),
                        )
                    nc.tensor.matmul(
                        pd, lh, ones_sb[:, 0:1],
                        start=(nt == 0), stop=(nt == NT - 1),
                    )
                recip = spool.tile([P, 1], f32, tag="recip", name=f"rc{mb}")
                nc.vector.reciprocal(recip[:], pd)
                osb = opool.tile([P, H], f32, tag="osb", name=f"osb{mb}")
                for ef in range(EF):
                    nc.vector.scalar_tensor_tensor(
                        osb[:, ef * 512:(ef + 1) * 512], po[ef], recip[:],
                        bv_sb[:, ef * 512:(ef + 1) * 512],
                        op0=mult, op1=add,
                    )
                nc.sync.dma_start(out_d.ap()[mb * P:(mb + 1) * P, :], osb[:])

    nc.compile()
    return nc


def kernel(hidden_states, attention_mask, Wq, bq, Wk, bk, Wv, bv):
    from concourse.bass_utils import run_bass_kernel_spmd

    nc = _build()

    hs = np.asarray(hidden_states, dtype=np.float32)
    mask = np.asarray(attention_mask, dtype=np.float32)
    wq32 = np.asarray(Wq, dtype=np.float32)
    wk32 = np.asarray(Wk, dtype=np.float32)
    bq32 = np.asarray(bq, dtype=np.float32).reshape(H)
    # scores = q.k = h_m^T (Wq Wk^T) h_n + (Wk bq).h_n + [per-row consts that
    # cancel in softmax]. G is precomputed; the z term folds into the mask.
    g16 = np.ascontiguousarray((wq32 @ wk32.T).astype(np.float16))
    u = wk32 @ bq32                      # [H]
    z = hs @ u                           # [B, S]
    wv16 = np.ascontiguousarray(np.asarray(Wv, dtype=np.float32).astype(np.float16))
    bv32 = np.ascontiguousarray(
        np.broadcast_to(np.asarray(bv, dtype=np.float32).reshape(1, H), (P, H))
    )

    in_maps = []
    for c in range(N_CORES):
        b, h = divmod(c, 2)
        qs = h * M
        if h == 0:
            order = np.arange(S)
        else:
            order = np.concatenate([np.arange(M, S), np.arange(0, M)])
        hs_r = hs[b][order]  # [S, H], own queries first
        hsT_c = np.ascontiguousarray(hs_r.T.astype(np.float16))  # [H, S]
        mask_c = np.ascontiguousarray(
            32.0 * (mask[b, qs:qs + M][:, order].T + z[b][order][:, None])
        ).astype(np.float16)
        in_maps.append(
            {
                "hsT": hsT_c,
                "g": g16,
                "wv": wv16,
                "bv": bv32,
                "mask": mask_c,
            }
        )

    res = run_bass_kernel_spmd(
        nc, in_maps, core_ids=list(range(N_CORES)), trace=TRACE
    )
    if TRACE:
        kernel.last_results = res

    out = np.empty((B, S, H), dtype=np.float32)
    for c in range(N_CORES):
        b, h = divmod(c, 2)
        out[b, h * M:(h + 1) * M] = res.results[c]["out"]
    return out
